# revision 16
# baseline (speedup 1.0000x reference)
"""Block-sparse matmul kernel for Trainium2 (8 NeuronCores, SPMD).

Problem: out = relu(x @ W_sparse + bias)
  x      [1024, 4096] f32
  kernel [4096, 32, 32] f32   (active 32x32 blocks)
  bias   [4096] f32
  ci, co [4096] int32         (block-row / block-col of each active block)
  out    [1024, 4096] f32

Strategy (v1, dense): scatter blocks into a dense [4096, 4096] weight
matrix on the host, cast x/W to bf16, and run a dense GEMM sharded
2-way over batch x 4-way over output columns (8 cores).  Each core
computes outT = W_slab.T @ x_half.T in [out, batch] orientation so
bias becomes a per-partition scalar for the ScalarE activation
(fused bias + relu straight out of PSUM).  The Bass program is
identical on all cores; only the data differs (SPMD-safe).
"""

import numpy as np
import ml_dtypes

import concourse.bacc as bacc
import concourse.bass as bass
import concourse.mybir as mybir
import concourse.tile as tile
from concourse.bass_utils import run_bass_kernel_spmd

BS = 32
N_IN = 4096
N_OUT = 4096
BATCH = 1024
N_CORES = 8

# sharding grid: 4 output-column quarters x 2 batch halves
CO_SHARDS = 4
B_SHARDS = 2
M_PER_CORE = N_OUT // CO_SHARDS          # 1024 output cols per core
B_PER_CORE = BATCH // B_SHARDS           # 512 batch rows per core
N_MTILES = M_PER_CORE // 128             # 8
N_KTILES = N_IN // 128                   # 32

BF16 = mybir.dt.bfloat16
F32 = mybir.dt.float32

_CACHE = {}


def _build_program():
    """Dense GEMM program, one core's share: outT[m,p,b] = relu(sum_k
    W[k,128m+p] * x[b,k] + bias[128m+p]).  Identical on all cores."""
    nc = bacc.Bacc(trn_type="TRN2")

    xT_d = nc.dram_tensor("xT", [128, N_KTILES * B_PER_CORE], BF16,
                          kind="ExternalInput")
    # wK[p, k*1024 + m*128 + c] = Wdense[128k+p, 128m+c] (per-core slab)
    wK_d = nc.dram_tensor("wK", [128, N_KTILES * N_MTILES * 128], BF16,
                          kind="ExternalInput")
    bias_d = nc.dram_tensor("biasv", [128, N_MTILES], F32,
                            kind="ExternalInput")
    outT_d = nc.dram_tensor("outT", [N_MTILES, 128, B_PER_CORE], F32,
                            kind="ExternalOutput")

    MCOLS = N_MTILES * 128  # 1024 W cols per k-tile

    with tile.TileContext(nc) as tc:
        with (
            tc.tile_pool(name="xp", bufs=1) as xp,
            tc.tile_pool(name="wp", bufs=1) as wp,
            tc.tile_pool(name="bp", bufs=1) as bp,
            tc.tile_pool(name="op", bufs=8) as op,
            tc.tile_pool(name="ps", bufs=8, space="PSUM") as ps,
            tc.tile_pool(name="wu", bufs=1) as wu,
        ):
            accs = [ps.tile([128, B_PER_CORE], F32, tag="acc",
                            name=f"acc{m}")
                    for m in range(N_MTILES)]

            # --- HAM warmup: keep PE busy while the first DMAs land so
            # the real matmul stream starts at K=8/8 (2.4 GHz).
            wut = wu.tile([128, 128], BF16)
            nc.vector.memset(wut[:], 0.0)
            for _ in range(48):
                nc.tensor.matmul(accs[N_MTILES - 1][:, 0:128],
                                 wut[:], wut[:], start=True, stop=True)

            # x and W fully resident in SBUF, streamed in k-order chunks.
            # Long per-partition rows amortize the ~0.6-0.8us/row DMA
            # overhead; x goes on gpsimd+scalar queues, W on sync queues
            # so the streams run in parallel.
            xt = xp.tile([128, N_KTILES * B_PER_CORE], BF16)
            wt = wp.tile([128, N_KTILES * MCOLS], BF16)
            # first chunks small + partition-split for fast arrival
            XC0 = 8 * B_PER_CORE      # 8 k-tiles of x (1 MiB)
            nc.gpsimd.dma_start(xt[0:64, 0:XC0], xT_d[0:64, 0:XC0])
            nc.scalar.dma_start(xt[64:128, 0:XC0], xT_d[64:128, 0:XC0])
            WC0 = 4 * MCOLS           # 4 k-tiles of W (1 MiB)
            nc.sync.dma_start(wt[:, 0:WC0], wK_d[:, 0:WC0])
            for j in range(1, 4):     # rest of x: 1 MiB chunks, 8KiB rows
                xlo, xhi = j * XC0, (j + 1) * XC0
                eng = nc.gpsimd if j % 2 else nc.scalar
                eng.dma_start(xt[:, xlo:xhi], xT_d[:, xlo:xhi])
            wbounds = [4, 12, 20, 28, 32]  # k-tile chunk edges after WC0/4
            for j in range(len(wbounds) - 1):
                wlo, whi = wbounds[j] * MCOLS, wbounds[j + 1] * MCOLS
                nc.sync.dma_start(wt[:, wlo:whi], wK_d[:, wlo:whi])

            bv = bp.tile([128, N_MTILES], F32)
            nc.sync.dma_start(bv[:], bias_d[:])

            # k-outer / m-inner: all 8 PSUM banks accumulate concurrently;
            # step k consumes only x[k] (128KiB) + W[k] (256KiB).
            for k in range(N_KTILES):
                for m in range(N_MTILES):
                    nc.tensor.matmul(
                        accs[m][:],
                        wt[:, k * MCOLS + m * 128: k * MCOLS + (m + 1) * 128],
                        xt[:, k * B_PER_CORE:(k + 1) * B_PER_CORE],
                        start=(k == 0),
                        stop=(k == N_KTILES - 1),
                    )

            for m in range(N_MTILES):
                ot = op.tile([128, B_PER_CORE], F32, tag="o")
                nc.scalar.activation(ot[:], accs[m][:],
                                     mybir.ActivationFunctionType.Relu,
                                     bias=bv[:, m:m + 1])
                nc.sync.dma_start(outT_d[m], ot[:])

    nc.compile()
    return nc


def _dense_weight(kernel_blocks, ci, co):
    """Scatter [N_BLK,32,32] blocks into dense [N_IN, N_OUT] (duplicates sum)."""
    nbr, nbc = N_IN // BS, N_OUT // BS
    wd4 = np.zeros((nbr, nbc, BS, BS), np.float32)
    np.add.at(wd4, (ci.astype(np.int64), co.astype(np.int64)),
              kernel_blocks.astype(np.float32))
    return wd4.transpose(0, 2, 1, 3).reshape(N_IN, N_OUT)


def _prep_inputs(x, kernel_blocks, bias, ci, co):
    x = np.asarray(x, np.float32)
    bias = np.asarray(bias, np.float32)
    ci = np.asarray(ci)
    co = np.asarray(co)
    wd = _dense_weight(np.asarray(kernel_blocks), ci, co)

    x_bf = x.astype(ml_dtypes.bfloat16)
    wd_bf = wd.astype(ml_dtypes.bfloat16)

    in_maps = []
    for c in range(N_CORES):
        q, h = divmod(c, B_SHARDS)
        xs = x_bf[h * B_PER_CORE:(h + 1) * B_PER_CORE]      # [512, 4096]
        # xT[p, k*512+b] = xs[b, 128k+p]
        xT = np.ascontiguousarray(
            xs.reshape(B_PER_CORE, N_KTILES, 128).transpose(2, 1, 0)
            .reshape(128, N_KTILES * B_PER_CORE))
        ws = wd_bf[:, q * M_PER_CORE:(q + 1) * M_PER_CORE]  # [4096, 1024]
        # wK[p, k*1024 + m*128 + cc] = ws[128k+p, 128m+cc]
        wK = np.ascontiguousarray(
            ws.reshape(N_KTILES, 128, N_MTILES * 128).transpose(1, 0, 2)
            .reshape(128, N_KTILES * N_MTILES * 128))
        bs = bias[q * M_PER_CORE:(q + 1) * M_PER_CORE]
        biasv = np.ascontiguousarray(bs.reshape(N_MTILES, 128).T)
        in_maps.append({"xT": xT, "wK": wK, "biasv": biasv})
    return in_maps


def _assemble(results):
    out = np.empty((BATCH, N_OUT), np.float32)
    for c in range(N_CORES):
        q, h = divmod(c, B_SHARDS)
        o = results[c]["outT"]  # [8, 128, 512] = [m, p, b]
        out[h * B_PER_CORE:(h + 1) * B_PER_CORE,
            q * M_PER_CORE:(q + 1) * M_PER_CORE] = (
            o.transpose(2, 0, 1).reshape(B_PER_CORE, M_PER_CORE))
    return out


def run(x, kernel, bias, ci, co, trace=False):
    if "nc" not in _CACHE:
        _CACHE["nc"] = _build_program()
    nc = _CACHE["nc"]
    in_maps = _prep_inputs(x, kernel, bias, ci, co)
    res = run_bass_kernel_spmd(nc, in_maps, core_ids=list(range(N_CORES)),
                               trace=trace)
    return _assemble(res.results), res


def kernel(x, kernel, bias, ci, co):
    out, _ = run(x, kernel, bias, ci, co, trace=False)
    return out
